# revision 1
# baseline (speedup 1.0000x reference)
"""GQA attention (B=1, T=2048, D=2048, H=32, KVH=8, HD=64) on 8 TRN2 cores.

Head-tensor-parallel: core c owns kv-head c and q-heads 4c..4c+3.
wq/wk/wv column-parallel, wo row-parallel; partials summed on host.
"""
import sys

if "/opt/trn_rl_repo" not in sys.path:
    sys.path.insert(0, "/opt/trn_rl_repo")

import numpy as np
import ml_dtypes

import concourse.bacc as bacc
import concourse.mybir as mybir
import concourse.tile as tile
from concourse.bass_utils import run_bass_kernel_spmd

BF16 = ml_dtypes.bfloat16
T, D, H, KVH, HD = 2048, 2048, 32, 8, 64
NCORES = 8
HPC = H // NCORES            # 4 q heads per core
KT, PT = 16, 128             # k-tiles of 128 over D
NCH = 4                      # t chunks of 512
CH = 512

_cache = {}


def _build_nc():
    if "nc" in _cache:
        return _cache["nc"]
    fp32, bf16 = mybir.dt.float32, mybir.dt.bfloat16
    Exp = mybir.ActivationFunctionType.Exp
    mult = mybir.AluOpType.mult
    nc = bacc.Bacc("TRN2", target_bir_lowering=False, debug=False,
                   num_devices=NCORES)

    xt_d = nc.dram_tensor("xt", [D, T], bf16, kind="ExternalInput")
    wq_d = nc.dram_tensor("wq", [D, HPC * HD], bf16, kind="ExternalInput")
    wkv_d = nc.dram_tensor("wkv", [D, 2 * HD], bf16, kind="ExternalInput")
    wo_d = nc.dram_tensor("wo", [HPC * HD, D], bf16, kind="ExternalInput")
    cs4_d = nc.dram_tensor("cs4", [PT, T], bf16, kind="ExternalInput")
    sn4_d = nc.dram_tensor("sn4", [PT, T], bf16, kind="ExternalInput")
    pe_d = nc.dram_tensor("permE", [PT, 2 * PT], bf16, kind="ExternalInput")
    po_d = nc.dram_tensor("permO", [PT, 2 * PT], bf16, kind="ExternalInput")
    id_d = nc.dram_tensor("ident", [PT, PT], bf16, kind="ExternalInput")
    mk_d = nc.dram_tensor("masks", [PT, 4, NCH * CH], bf16, kind="ExternalInput")
    out_d = nc.dram_tensor("partial", [T, D], bf16, kind="ExternalOutput")

    with tile.TileContext(nc) as tc:
        with tc.tile_pool(name="const", bufs=1) as const, \
             tc.tile_pool(name="xtp", bufs=KT) as xtp, \
             tc.tile_pool(name="persist", bufs=1) as persist:

            # ---- loads: small consts first, then xt stream, then wq/wo ----
            wkv_sb = const.tile([PT, KT, 2 * HD], bf16, tag="wkv")
            nc.sync.dma_start(wkv_sb[:], wkv_d.ap().rearrange("(k p) m -> p k m", p=PT))
            cs4 = const.tile([PT, T], bf16, tag="cs4")
            nc.sync.dma_start(cs4[:], cs4_d.ap())
            sn4 = const.tile([PT, T], bf16, tag="sn4")
            nc.sync.dma_start(sn4[:], sn4_d.ap())
            permE = const.tile([PT, 2 * PT], bf16, tag="permE")
            nc.sync.dma_start(permE[:], pe_d.ap())
            permO = const.tile([PT, 2 * PT], bf16, tag="permO")
            nc.sync.dma_start(permO[:], po_d.ap())
            ident = const.tile([PT, PT], bf16, tag="ident")
            nc.sync.dma_start(ident[:], id_d.ap())
            masks = const.tile([PT, 4, NCH * CH], bf16, tag="masks")
            nc.sync.dma_start(masks[:], mk_d.ap())
            xt = []
            for k in range(KT):
                t_ = xtp.tile([PT, T], bf16, tag="xt")
                nc.sync.dma_start(t_[:], xt_d.ap()[k * PT:(k + 1) * PT, :])
                xt.append(t_)
            wq_sb = const.tile([PT, KT, HPC * HD], bf16, tag="wq")
            nc.sync.dma_start(wq_sb[:], wq_d.ap().rearrange("(k p) m -> p k m", p=PT))
            wo_sb = const.tile([PT, 2, D], bf16, tag="wo")
            nc.sync.dma_start(wo_sb[:], wo_d.ap().rearrange("(s p) m -> p s m", p=PT))
            ones_v = const.tile([1, 1], bf16, tag="ones_v")
            nc.vector.memset(ones_v[:], 1.0)

            # persistent activations: qtc[j] = [h0|h1|h2|h3] qT for chunk j
            qtc = [persist.tile([64, HPC * CH], bf16, tag=f"qtc{j}", name=f"qtc{j}")
                   for j in range(NCH)]
            kt = persist.tile([64, T], bf16, tag="kt")
            vx = [persist.tile([PT, HD + 1], bf16, tag=f"vx{s}", name=f"vx{s}")
                  for s in range(KT)]
            ot = [persist.tile([PT, T], bf16, tag=f"ot{p}", name=f"ot{p}")
                  for p in range(2)]

            # ---- phase A: kv projection (k-outer, paced to xt arrivals) ----
            with tc.tile_pool(name="kvp", bufs=1, space="PSUM") as kvp, \
                 tc.tile_pool(name="vtrp", bufs=2, space="PSUM") as vtrp, \
                 tc.tile_pool(name="tmpa", bufs=2) as tmpa:
                KV = [kvp.tile([PT, CH], fp32, tag=f"kv{j}", name=f"kv{j}")
                      for j in range(NCH)]
                for k in range(KT):
                    for j in range(NCH):
                        nc.tensor.matmul(KV[j][:], wkv_sb[:, k, :],
                                         xt[k][:, j * CH:(j + 1) * CH],
                                         start=(k == 0), stop=(k == KT - 1))
                for j in range(NCH):
                    jsl = slice(j * CH, (j + 1) * CH)
                    k1 = tmpa.tile([32, CH], fp32, tag="k1")
                    k2 = tmpa.tile([32, CH], fp32, tag="k2")
                    nc.vector.tensor_tensor(k1[:], KV[j][0:32, :], cs4[0:32, jsl], mult)
                    nc.vector.tensor_tensor(k2[:], KV[j][32:64, :], sn4[0:32, jsl], mult)
                    nc.vector.tensor_sub(kt[0:32, jsl], k1[:], k2[:])
                    k3 = tmpa.tile([32, CH], fp32, tag="k1")
                    k4 = tmpa.tile([32, CH], fp32, tag="k2")
                    nc.vector.tensor_tensor(k3[:], KV[j][0:32, :], sn4[0:32, jsl], mult)
                    nc.vector.tensor_tensor(k4[:], KV[j][32:64, :], cs4[0:32, jsl], mult)
                    nc.vector.tensor_add(kt[32:64, jsl], k3[:], k4[:])
                    vt = tmpa.tile([64, CH], bf16, tag="vt")
                    nc.vector.tensor_copy(vt[:], KV[j][64:PT, :])
                    for u in range(4):
                        s_idx = 4 * j + u
                        vtr = vtrp.tile([PT, 64], bf16, tag="vtr")
                        nc.tensor.transpose(vtr[:], vt[:, u * PT:(u + 1) * PT],
                                            ident[:64, :64])
                        nc.vector.tensor_copy(vx[s_idx][:, 0:HD], vtr[:])
                        nc.vector.memset(vx[s_idx][:, HD:HD + 1], 1.0)

            # ---- phase B: q projection + rope + repack ----
            with tc.tile_pool(name="qe", bufs=2, space="PSUM") as qep, \
                 tc.tile_pool(name="qpp", bufs=2, space="PSUM") as qpp, \
                 tc.tile_pool(name="tmpb", bufs=2) as tmpb:
                for j in range(NCH):
                    jsl = slice(j * CH, (j + 1) * CH)
                    E = qep.tile([PT, CH], fp32, tag="E")
                    O = qep.tile([PT, CH], fp32, tag="O")
                    for k in range(KT):
                        st, sp = (k == 0), (k == KT - 1)
                        nc.tensor.matmul(E[:], wq_sb[:, k, 0:PT], xt[k][:, jsl],
                                         start=st, stop=sp)
                        nc.tensor.matmul(O[:], wq_sb[:, k, PT:2 * PT], xt[k][:, jsl],
                                         start=st, stop=sp)
                    t1 = tmpb.tile([PT, CH], fp32, tag="t1")
                    t2 = tmpb.tile([PT, CH], fp32, tag="t2")
                    rE = tmpb.tile([PT, CH], bf16, tag="rE")
                    rO = tmpb.tile([PT, CH], bf16, tag="rO")
                    nc.vector.tensor_tensor(t1[:], E[:], cs4[:, jsl], mult)
                    nc.vector.tensor_tensor(t2[:], O[:], sn4[:, jsl], mult)
                    nc.vector.tensor_sub(rE[:], t1[:], t2[:])
                    t3 = tmpb.tile([PT, CH], fp32, tag="t1")
                    t4 = tmpb.tile([PT, CH], fp32, tag="t2")
                    nc.vector.tensor_tensor(t3[:], E[:], sn4[:, jsl], mult)
                    nc.vector.tensor_tensor(t4[:], O[:], cs4[:, jsl], mult)
                    nc.vector.tensor_add(rO[:], t3[:], t4[:])
                    for h in range(HPC):
                        qp = qpp.tile([64, CH], fp32, tag="qp")
                        nc.tensor.matmul(qp[:], permE[:, 64 * h:64 * h + 64],
                                         rE[:], start=True, stop=False)
                        nc.tensor.matmul(qp[:], permO[:, 64 * h:64 * h + 64],
                                         rO[:], start=False, stop=True)
                        nc.vector.tensor_copy(qtc[j][:, h * CH:(h + 1) * CH], qp[:])

            # ---- phase C: attention (4-head quad tiles per (i, j)) ----
            with tc.tile_pool(name="sc", bufs=1, space="PSUM") as scp, \
                 tc.tile_pool(name="pv", bufs=1, space="PSUM") as pvp, \
                 tc.tile_pool(name="ex", bufs=3) as exp_pool, \
                 tc.tile_pool(name="nrm", bufs=2) as nrm:
                for j in range(NCH):
                    pv = [pvp.tile([HD + 1, CH], fp32, tag=f"pv{h}", name=f"pv{h}_{j}")
                          for h in range(HPC)]
                    for i in range(4 * j + 4):
                        ktsl = kt[:, i * PT:(i + 1) * PT]
                        sc = scp.tile([PT, HPC * CH], fp32, tag="sc")
                        for h in range(HPC):
                            nc.tensor.matmul(sc[:, h * CH:(h + 1) * CH], ktsl,
                                             qtc[j][:, h * CH:(h + 1) * CH],
                                             start=True, stop=True)
                        ex = exp_pool.tile([PT, HPC * CH], bf16, tag="ex")
                        nc.scalar.activation(ex[:], sc[:], Exp, scale=0.125)
                        if i // 4 == j:
                            nc.gpsimd.tensor_tensor(ex[:], ex[:],
                                                    masks[:, i % 4, :], mult)
                        for h in range(HPC):
                            nc.tensor.matmul(pv[h][:], vx[i],
                                             ex[:, h * CH:(h + 1) * CH],
                                             start=(i == 0), stop=(i == 4 * j + 3))
                    for h in range(HPC):
                        srow = nrm.tile([1, CH], fp32, tag="srow")
                        nc.vector.tensor_copy(srow[:], pv[h][HD:HD + 1, :])
                        rrow = nrm.tile([1, CH], fp32, tag="rrow")
                        nc.vector.reciprocal_approx_fast(rrow[:], srow[:])
                        bc = nrm.tile([64, CH], fp32, tag="bc")
                        nc.gpsimd.partition_broadcast(bc[:], rrow[:])
                        nc.vector.tensor_tensor(
                            ot[h // 2][64 * (h % 2):64 * (h % 2) + 64,
                                       j * CH:(j + 1) * CH],
                            pv[h][0:HD, :], bc[:], mult)

            # ---- phase D: output projection ----
            with tc.tile_pool(name="wp", bufs=4, space="PSUM") as wpp, \
                 tc.tile_pool(name="po", bufs=4) as pop:
                n = 0
                for tt in range(KT):
                    for dd in range(NCH):
                        wp = wpp.tile([PT, CH], fp32, tag="wp")
                        for s in range(2):
                            nc.tensor.matmul(wp[:], ot[s][:, tt * PT:(tt + 1) * PT],
                                             wo_sb[:, s, dd * CH:(dd + 1) * CH],
                                             start=(s == 0), stop=(s == 1))
                        pout = pop.tile([PT, CH], bf16, tag="po")
                        if n % 2 == 0:
                            nc.scalar.copy(pout[:], wp[:])
                        else:
                            nc.vector.tensor_copy(pout[:], wp[:])
                        n += 1
                        nc.sync.dma_start(
                            out_d.ap()[tt * PT:(tt + 1) * PT, dd * CH:(dd + 1) * CH],
                            pout[:])

    nc.compile()
    _cache["nc"] = nc
    return nc


def _host_prep(x, freqs, wq, wk, wv, wo):
    x2d = np.asarray(x, np.float32)[0]                    # [T, D]
    xt = np.ascontiguousarray(x2d.T).astype(BF16)         # [D, T]
    cos = np.cos(np.asarray(freqs, np.float32))           # [T, 32]
    sin = np.sin(np.asarray(freqs, np.float32))
    cs4 = np.ascontiguousarray(np.tile(cos.T, (4, 1)))    # [128, T]
    sn4 = np.ascontiguousarray(np.tile(sin.T, (4, 1)))

    ev, od = np.arange(0, HD, 2), np.arange(1, HD, 2)

    # permE/permO [128, 256]: head h (cols 64h..64h+63): local row r<32 comes
    # from rE row 32h+r, r>=32 from rO row 32h+(r-32)
    permE = np.zeros((PT, 2 * PT), np.float32)
    permO = np.zeros((PT, 2 * PT), np.float32)
    for h in range(HPC):
        for r in range(32):
            permE[32 * h + r, 64 * h + r] = 1.0
            permO[32 * h + r, 64 * h + 32 + r] = 1.0

    ident = np.eye(PT, dtype=np.float32)

    # masks[sig, r, :] tiled x4 for the 4-head quad layout
    m1 = np.zeros((PT, 4, CH), np.float32)
    sig = np.arange(PT)[:, None]
    kap = np.arange(CH)[None, :]
    for r in range(4):
        m1[:, r, :] = (kap >= sig + PT * r).astype(np.float32)
    masks = np.ascontiguousarray(np.tile(m1, (1, 1, HPC)))  # [128, 4, 2048]

    wq_f = np.asarray(wq, np.float32)
    wk_f = np.asarray(wk, np.float32)
    wv_f = np.asarray(wv, np.float32)
    wo_f = np.asarray(wo, np.float32)

    in_maps = []
    for c in range(NCORES):
        # wq for 4 heads, evens-major-across-heads packing:
        # cols 0:128 = [h0 evens, h1 evens, h2 evens, h3 evens], 128:256 odds
        blocks = [wq_f[:, (c * HPC + h) * HD:(c * HPC + h + 1) * HD] for h in range(HPC)]
        wq_c = np.concatenate([b[:, ev] for b in blocks] + [b[:, od] for b in blocks], axis=1)
        kblk = wk_f[:, c * HD:(c + 1) * HD]
        wkv_c = np.concatenate([kblk[:, ev], kblk[:, od],
                                wv_f[:, c * HD:(c + 1) * HD]], axis=1)
        wo_c = wo_f[c * HPC * HD:(c + 1) * HPC * HD, :]
        in_maps.append({
            "xt": xt,
            "wq": np.ascontiguousarray(wq_c).astype(BF16),
            "wkv": np.ascontiguousarray(wkv_c).astype(BF16),
            "wo": np.ascontiguousarray(wo_c).astype(BF16),
            "cs4": cs4.astype(BF16),
            "sn4": sn4.astype(BF16),
            "permE": permE.astype(BF16),
            "permO": permO.astype(BF16),
            "ident": ident.astype(BF16),
            "masks": masks.astype(BF16),
        })
    return in_maps


def run(inputs, trace=False, tmpdir=None):
    nc = _build_nc()
    in_maps = _host_prep(**inputs)
    res = run_bass_kernel_spmd(nc, in_maps, list(range(NCORES)),
                               trace=trace, tmpdir=tmpdir)
    acc = np.zeros((T, D), np.float32)
    for c in range(NCORES):
        acc += res.results[c]["partial"].astype(np.float32)
    return acc[None], res


def kernel(**inputs):
    out, _ = run(inputs, trace=False)
    return out



# revision 6
# speedup vs baseline: 1.5444x; 1.5444x over previous
"""GQA attention (B=1, T=2048, D=2048, H=32, KVH=8, HD=64) on 8 TRN2 cores.

Head-tensor-parallel: core c owns kv-head c and q-heads 4c..4c+3.
wq/wk/wv column-parallel, wo row-parallel; partials summed on host.

Pipelined layout: A (kv proj + rope-k + v transpose) interleaved with
B (q proj + rope-q + perm) chunk by chunk, then C (attention) with
score/pv matmuls software-pipelined against ACT exp, then D (wo proj).
"""
import sys

if "/opt/trn_rl_repo" not in sys.path:
    sys.path.insert(0, "/opt/trn_rl_repo")

import numpy as np
import ml_dtypes

import concourse.bacc as bacc
import concourse.mybir as mybir
import concourse.tile as tile
from concourse.bass_utils import run_bass_kernel_spmd

BF16 = ml_dtypes.bfloat16
T, D, H, KVH, HD = 2048, 2048, 32, 8, 64
NCORES = 8
HPC = H // NCORES            # 4 q heads per core
KT, PT = 16, 128             # k-tiles of 128 over D
NCH = 4                      # t chunks of 512
CH = 512

_cache = {}


def _build_nc():
    if "nc" in _cache:
        return _cache["nc"]
    fp32, bf16 = mybir.dt.float32, mybir.dt.bfloat16
    Exp = mybir.ActivationFunctionType.Exp
    mult = mybir.AluOpType.mult
    nc = bacc.Bacc("TRN2", target_bir_lowering=False, debug=False,
                   num_devices=NCORES)

    xt_d = nc.dram_tensor("xt", [D, T], bf16, kind="ExternalInput")
    wq_d = nc.dram_tensor("wq", [D, HPC * HD], bf16, kind="ExternalInput")
    wkv_d = nc.dram_tensor("wkv", [D, 2 * HD], bf16, kind="ExternalInput")
    wo_d = nc.dram_tensor("wo", [HPC * HD, D], bf16, kind="ExternalInput")
    cs4_d = nc.dram_tensor("cs4", [PT, T], bf16, kind="ExternalInput")
    sn4_d = nc.dram_tensor("sn4", [PT, T], bf16, kind="ExternalInput")
    pe_d = nc.dram_tensor("permE", [PT, 2 * PT], bf16, kind="ExternalInput")
    po_d = nc.dram_tensor("permO", [PT, 2 * PT], bf16, kind="ExternalInput")
    id_d = nc.dram_tensor("ident", [64, 64], bf16, kind="ExternalInput")
    tri_d = nc.dram_tensor("tri4", [PT, HPC, PT], bf16, kind="ExternalInput")
    out_d = nc.dram_tensor("partial", [T, D], bf16, kind="ExternalOutput")

    with tile.TileContext(nc) as tc:
        with tc.tile_pool(name="const", bufs=1) as const, \
             tc.tile_pool(name="xtp", bufs=4 * KT) as xtp, \
             tc.tile_pool(name="persist", bufs=1) as persist:

            # ---- DMA loads, priority order ----
            wkv_sb = const.tile([PT, KT, 2 * HD], bf16, tag="wkv")
            nc.sync.dma_start(wkv_sb[:], wkv_d.ap().rearrange("(k p) m -> p k m", p=PT))
            xt = [[None] * KT for _ in range(NCH)]
            for k in range(4):
                t_ = xtp.tile([PT, CH], bf16, tag="xt", name=f"xt_{0}_{k}")
                nc.sync.dma_start(t_[:], xt_d.ap()[k * PT:(k + 1) * PT, 0:CH])
                xt[0][k] = t_
            wq_sb = const.tile([PT, KT, HPC * HD], bf16, tag="wq")
            nc.sync.dma_start(wq_sb[:], wq_d.ap().rearrange("(k p) m -> p k m", p=PT))
            for k in range(4, KT):
                t_ = xtp.tile([PT, CH], bf16, tag="xt", name=f"xt_{0}_{k}")
                nc.sync.dma_start(t_[:], xt_d.ap()[k * PT:(k + 1) * PT, 0:CH])
                xt[0][k] = t_
            cs4 = const.tile([PT, T], bf16, tag="cs4")
            nc.sync.dma_start(cs4[:], cs4_d.ap())
            sn4 = const.tile([PT, T], bf16, tag="sn4")
            nc.sync.dma_start(sn4[:], sn4_d.ap())
            permE = const.tile([PT, 2 * PT], bf16, tag="permE")
            nc.sync.dma_start(permE[:], pe_d.ap())
            permO = const.tile([PT, 2 * PT], bf16, tag="permO")
            nc.sync.dma_start(permO[:], po_d.ap())
            ident = const.tile([64, 64], bf16, tag="ident")
            nc.sync.dma_start(ident[:], id_d.ap())
            tri4 = const.tile([PT, HPC, PT], bf16, tag="tri4")
            nc.sync.dma_start(tri4[:], tri_d.ap())
            for j in range(1, NCH):
                for k in range(KT):
                    t_ = xtp.tile([PT, CH], bf16, tag="xt", name=f"xt_{j}_{k}")
                    nc.sync.dma_start(
                        t_[:], xt_d.ap()[k * PT:(k + 1) * PT, j * CH:(j + 1) * CH])
                    xt[j][k] = t_
            wo_sb = const.tile([PT, 2, D], bf16, tag="wo")
            nc.sync.dma_start(wo_sb[:], wo_d.ap().rearrange("(s p) m -> p s m", p=PT))

            # ---- persistent SBUF activations ----
            kt = persist.tile([64, T], bf16, tag="kt")
            vx = persist.tile([PT, KT, HD + 1], bf16, tag="vx")
            nc.vector.memset(vx[:, :, HD:HD + 1], 1.0)
            qtc = [persist.tile([64, HPC * CH], bf16, tag=f"qtc{j}", name=f"qtc{j}")
                   for j in range(NCH)]
            ot = [persist.tile([PT, T], bf16, tag=f"ot{s}", name=f"ot{s}")
                  for s in range(2)]

            # ---- phase A || B: projections + rope, chunk by chunk ----
            # PSUM: proj pool (KV,E,O rotate in 2 banks via bufs=2) would
            # stall; use KV bufs=2 (2), EO bufs=2x2 (4), vtr 1, qp 1 = 8.
            with tc.tile_pool(name="kvp", bufs=2, space="PSUM") as kvp, \
                 tc.tile_pool(name="eop", bufs=2, space="PSUM") as eop, \
                 tc.tile_pool(name="vtp", bufs=1, space="PSUM") as vtp, \
                 tc.tile_pool(name="qpp", bufs=1, space="PSUM") as qpp, \
                 tc.tile_pool(name="tmpab", bufs=2) as tmpab:
                rope_q = {}

                def emit_perm(jj):
                    # perm (PE) -> qp psum -> qtc copies (ACT)
                    rE, rO = rope_q.pop(jj)
                    for h in range(HPC):
                        qp = qpp.tile([64, CH], fp32, tag="qp")
                        nc.tensor.matmul(qp[:], permE[:, 64 * h:64 * h + 64],
                                         rE[:], start=True, stop=False)
                        nc.tensor.matmul(qp[:], permO[:, 64 * h:64 * h + 64],
                                         rO[:], start=False, stop=True)
                        nc.scalar.copy(qtc[jj][:, h * CH:(h + 1) * CH], qp[:])

                for j in range(NCH):
                    jsl = slice(j * CH, (j + 1) * CH)
                    # A(j): kv projection
                    KV = kvp.tile([PT, CH], fp32, tag="kv", name=f"kv{j}")
                    for k in range(KT):
                        nc.tensor.matmul(KV[:], wkv_sb[:, k, :], xt[j][k][:],
                                         start=(k == 0), stop=(k == KT - 1))
                    # rope-k (DVE): kt rows 0:32 = Kev*c - Kod*s ; 32:64 = Kev*s + Kod*c
                    k1 = tmpab.tile([32, CH], fp32, tag="k1")
                    k2 = tmpab.tile([32, CH], fp32, tag="k2")
                    nc.vector.tensor_tensor(k1[:], KV[0:32, :], cs4[0:32, jsl], mult)
                    nc.vector.tensor_tensor(k2[:], KV[32:64, :], sn4[0:32, jsl], mult)
                    nc.vector.tensor_sub(kt[0:32, jsl], k1[:], k2[:])
                    k3 = tmpab.tile([32, CH], fp32, tag="k1")
                    k4 = tmpab.tile([32, CH], fp32, tag="k2")
                    nc.vector.tensor_tensor(k3[:], KV[0:32, :], sn4[0:32, jsl], mult)
                    nc.vector.tensor_tensor(k4[:], KV[32:64, :], cs4[0:32, jsl], mult)
                    nc.vector.tensor_add(kt[32:64, jsl], k3[:], k4[:])
                    # v extract: vt copy on ACT (runs during E matmuls)
                    vt = tmpab.tile([64, CH], bf16, tag="vt")
                    nc.scalar.copy(vt[:], KV[64:PT, :])

                    # perm for previous chunk fills the rope-q(j-1) tail
                    if j >= 1:
                        emit_perm(j - 1)

                    # B(j): q projection E half
                    E = eop.tile([PT, CH], fp32, tag="E", name=f"E{j}")
                    O = eop.tile([PT, CH], fp32, tag="O", name=f"O{j}")
                    for k in range(KT):
                        st, sp = (k == 0), (k == KT - 1)
                        nc.tensor.matmul(E[:], wq_sb[:, k, 0:PT], xt[j][k][:],
                                         start=st, stop=sp)
                    # v transpose (PE, vt ready by now), vtr -> vx on gpsimd
                    vtr = vtp.tile([PT, 4, HD], bf16, tag="vtr")
                    for u in range(4):
                        nc.tensor.transpose(vtr[:, u, :], vt[:, u * PT:(u + 1) * PT],
                                            ident[:])
                    nc.vector.tensor_copy(vx[:, 4 * j:4 * j + 4, 0:HD], vtr[:])
                    # rope-q E-dependent part (DVE)
                    t1 = tmpab.tile([PT, CH], fp32, tag="t1")
                    t3 = tmpab.tile([PT, CH], fp32, tag="t3")
                    nc.vector.tensor_tensor(t1[:], E[:], cs4[:, jsl], mult)
                    nc.vector.tensor_tensor(t3[:], E[:], sn4[:, jsl], mult)
                    # B(j): q projection O half
                    for k in range(KT):
                        st, sp = (k == 0), (k == KT - 1)
                        nc.tensor.matmul(O[:], wq_sb[:, k, PT:2 * PT], xt[j][k][:],
                                         start=st, stop=sp)
                    # rope-q rest -> rE/rO bf16
                    t2 = tmpab.tile([PT, CH], fp32, tag="t2")
                    t4 = tmpab.tile([PT, CH], fp32, tag="t4")
                    rE = tmpab.tile([PT, CH], bf16, tag="rE")
                    rO = tmpab.tile([PT, CH], bf16, tag="rO")
                    nc.vector.tensor_tensor(t2[:], O[:], sn4[:, jsl], mult)
                    nc.vector.tensor_sub(rE[:], t1[:], t2[:])
                    nc.vector.tensor_tensor(t4[:], O[:], cs4[:, jsl], mult)
                    nc.vector.tensor_add(rO[:], t3[:], t4[:])
                    rope_q[j] = (rE, rO)
                emit_perm(NCH - 1)

            # ---- phase C: attention, software-pipelined ----
            # PSUM: scA/scB [128,2,512] = 2+2 banks, pv [65,4,512] = 4 banks.
            # Two head-pair streams so exp(pairA) overlaps scores(pairB); pv
            # matmuls lag one i behind scores. Per chunk j: diagonal blocks
            # first (col-restricted to the causal-live columns).
            with tc.tile_pool(name="scp", bufs=2, space="PSUM") as scp, \
                 tc.tile_pool(name="pvp", bufs=1, space="PSUM") as pvp, \
                 tc.tile_pool(name="exq", bufs=2) as exq, \
                 tc.tile_pool(name="nrm", bufs=2) as nrm:
                scg = [scp.tile([PT, 2, CH], fp32, tag="sc", name=f"sc{g}")
                       for g in range(2)]
                pv = pvp.tile([HD + 1, HPC, CH], fp32, tag="pv")
                for j in range(NCH):
                    idxs = list(range(4 * j, 4 * j + 4)) + list(range(4 * j))
                    nlast = len(idxs) - 1
                    pend = []  # (i, c0, [exA, exB], idx)

                    def flush_pv(nl=nlast):
                        i, c0, exg, idx = pend.pop(0)
                        for h in range(HPC):
                            nc.tensor.matmul(
                                pv[:, h, c0:CH], vx[:, i, :],
                                exg[h // 2][:, h % 2, c0:CH],
                                start=(idx == 0), stop=(idx == nl))

                    for idx, i in enumerate(idxs):
                        r = i - 4 * j if i >= 4 * j else None
                        c0 = PT * r if r is not None else 0
                        ktsl = kt[:, i * PT:(i + 1) * PT]
                        exg = []
                        for g in range(2):
                            for hh in range(2):
                                h = 2 * g + hh
                                nc.tensor.matmul(
                                    scg[g][:, hh, c0:CH], ktsl,
                                    qtc[j][:, h * CH + c0:(h + 1) * CH],
                                    start=True, stop=True)
                            ex = exq.tile([PT, 2, CH], bf16, tag=f"ex{g}",
                                          name=f"ex{g}")
                            nc.scalar.activation(ex[:, :, c0:CH],
                                                 scg[g][:, :, c0:CH],
                                                 Exp, scale=0.125)
                            if r is not None:
                                nc.vector.tensor_tensor(
                                    ex[:, :, c0:c0 + PT], ex[:, :, c0:c0 + PT],
                                    tri4[:, 2 * g:2 * g + 2, :], mult)
                            exg.append(ex)
                        pend.append((i, c0, exg, idx))
                        if len(pend) > 1:
                            flush_pv()
                    while pend:
                        flush_pv()
                    # normalization: recip(sums) -> broadcast -> scale into ot
                    srow = nrm.tile([1, HPC, CH], fp32, tag="srow")
                    nc.vector.tensor_copy(srow[:], pv[HD:HD + 1, :, :])
                    rr = nrm.tile([1, HPC, CH], fp32, tag="rr")
                    nc.vector.reciprocal_approx_fast(rr[:], srow[:])
                    for h in range(HPC):
                        bc = nrm.tile([64, CH], fp32, tag="bc")
                        nc.gpsimd.partition_broadcast(bc[:], rr[:, h, :])
                        nc.vector.tensor_tensor(
                            ot[h // 2][64 * (h % 2):64 * (h % 2) + 64,
                                       j * CH:(j + 1) * CH],
                            pv[0:HD, h, :], bc[:], mult)

            # ---- phase D: output projection ----
            with tc.tile_pool(name="wp", bufs=4, space="PSUM") as wpp, \
                 tc.tile_pool(name="po", bufs=4) as pop:
                n = 0
                for tt in range(KT):
                    for dd in range(NCH):
                        wp = wpp.tile([PT, CH], fp32, tag="wp")
                        for s in range(2):
                            nc.tensor.matmul(wp[:], ot[s][:, tt * PT:(tt + 1) * PT],
                                             wo_sb[:, s, dd * CH:(dd + 1) * CH],
                                             start=(s == 0), stop=(s == 1))
                        pout = pop.tile([PT, CH], bf16, tag="po")
                        if n % 2 == 0:
                            nc.scalar.copy(pout[:], wp[:])
                        else:
                            nc.vector.tensor_copy(pout[:], wp[:])
                        n += 1
                        nc.sync.dma_start(
                            out_d.ap()[tt * PT:(tt + 1) * PT, dd * CH:(dd + 1) * CH],
                            pout[:])

    nc.compile()
    _cache["nc"] = nc
    return nc


def _host_prep(x, freqs, wq, wk, wv, wo):
    x2d = np.asarray(x, np.float32)[0]                    # [T, D]
    xt = np.ascontiguousarray(x2d.T).astype(BF16)         # [D, T]
    cos = np.cos(np.asarray(freqs, np.float32))           # [T, 32]
    sin = np.sin(np.asarray(freqs, np.float32))
    cs4 = np.ascontiguousarray(np.tile(cos.T, (4, 1)))    # [128, T]
    sn4 = np.ascontiguousarray(np.tile(sin.T, (4, 1)))

    ev, od = np.arange(0, HD, 2), np.arange(1, HD, 2)

    # permE/permO [128, 256]: head h (cols 64h..64h+63): local row r<32 comes
    # from rE row 32h+r, r>=32 from rO row 32h+(r-32)
    permE = np.zeros((PT, 2 * PT), np.float32)
    permO = np.zeros((PT, 2 * PT), np.float32)
    for h in range(HPC):
        for r in range(32):
            permE[32 * h + r, 64 * h + r] = 1.0
            permO[32 * h + r, 64 * h + 32 + r] = 1.0

    ident = np.eye(64, dtype=np.float32)

    # tri4 [128, 4, 128]: causal triangle, tiled per head: keep col >= row
    sig = np.arange(PT)[:, None]
    kap = np.arange(PT)[None, :]
    tri = (kap >= sig).astype(np.float32)                 # [128, 128]
    tri4 = np.ascontiguousarray(
        np.broadcast_to(tri[:, None, :], (PT, HPC, PT)))

    wq_f = np.asarray(wq, np.float32)
    wk_f = np.asarray(wk, np.float32)
    wv_f = np.asarray(wv, np.float32)
    wo_f = np.asarray(wo, np.float32)

    in_maps = []
    for c in range(NCORES):
        # wq for 4 heads, evens-major-across-heads packing:
        # cols 0:128 = [h0 evens, h1 evens, h2 evens, h3 evens], 128:256 odds
        blocks = [wq_f[:, (c * HPC + h) * HD:(c * HPC + h + 1) * HD] for h in range(HPC)]
        wq_c = np.concatenate([b[:, ev] for b in blocks] + [b[:, od] for b in blocks], axis=1)
        kblk = wk_f[:, c * HD:(c + 1) * HD]
        wkv_c = np.concatenate([kblk[:, ev], kblk[:, od],
                                wv_f[:, c * HD:(c + 1) * HD]], axis=1)
        wo_c = wo_f[c * HPC * HD:(c + 1) * HPC * HD, :]
        in_maps.append({
            "xt": xt,
            "wq": np.ascontiguousarray(wq_c).astype(BF16),
            "wkv": np.ascontiguousarray(wkv_c).astype(BF16),
            "wo": np.ascontiguousarray(wo_c).astype(BF16),
            "cs4": cs4.astype(BF16),
            "sn4": sn4.astype(BF16),
            "permE": permE.astype(BF16),
            "permO": permO.astype(BF16),
            "ident": ident.astype(BF16),
            "tri4": tri4.astype(BF16),
        })
    return in_maps


def run(inputs, trace=False, tmpdir=None):
    nc = _build_nc()
    in_maps = _host_prep(**inputs)
    res = run_bass_kernel_spmd(nc, in_maps, list(range(NCORES)),
                               trace=trace, tmpdir=tmpdir)
    acc = np.zeros((T, D), np.float32)
    for c in range(NCORES):
        acc += res.results[c]["partial"].astype(np.float32)
    return acc[None], res


def kernel(**inputs):
    out, _ = run(inputs, trace=False)
    return out
